# revision 16
# baseline (speedup 1.0000x reference)
"""HGNN+ (2x HGNNPConv) Trainium2 kernel, 8-core SPMD, fp8 DoubleRow.

Strategy: the hypergraph v2v mean aggregation is a linear operator
    v2v(X) = Dv^-1 H De^-1 H^T X
with H the [N, E] incidence-count matrix. We shard vertices across the
8 cores and run the network as a chain of dense matmuls on the
TensorEngine, with the big contractions in fp8e4 DoubleRow perf mode
(two k-rows per PE cell -> ~2x FLOP rate):

  per core l (NL = N/8 = 2048 local vertices):
    M1  = relu(Xl @ W1) / 32            [NL, CH]   fp8 DR (W1 prescaled x32)
    E1p = H_l^T M1 * 1/de - mu1/8       [E, CH]    fp8 DR + fp32 scales
    E1r = AllReduce(E1p)                [E, CH]    fp8 (x16 residual vs mu1)
    V1t = relu(H E1r + dv (x) mu1)^T    [CH, NL]   fp8 DR + DVE rank-1 addback
    M2r = relu(V1 @ W2)*  1/dv - mu2    [NL, C2]   fp16 matmuls (accuracy)
    E2p = H_l^T M2r * 1/de              [E, C2]    fp8 DR
    E2r = AllReduce(E2p)                [E, C2]    fp8 (x16 residual vs mu2)
    OUT = (H E2r + dv (x) mu2) * 1/dv   [NL, C2]   fp8 DR, fp32 out

Precision design: fp8 RNE quantization of the near-rank-one activations
(E1/M2/E2) produces rounding errors that are correlated across the
segment-mean averaging and would NOT wash out (~3% final error). So H
is kept as raw 0/1 counts (exact in fp8), all 1/de & 1/dv normalizers
are exact fp32 per-partition scales at PSUM->SBUF copies, and E1/M2/E2
are quantized as residuals against a host-sampled mean estimate mu1
(mu2 = relu(mu1@W2)), whose rank-one contribution is carried exactly
and re-added on the (otherwise idle) DVE. W1 is prescaled by 32 so its
entries avoid the fp8 subnormal range. Measured ~2.4e-3 rel err.

Pipelining mirrors the fp16 predecessor: layer 1 runs per 512-wide
channel half with the half-0 AllReduce hidden under half-1 compute and
the half-1 AllReduce hidden under step 3's first output rows; layer 2
chunks its AllReduce over edge-row halves with partials parked in SBUF.
"""

import numpy as np
import ml_dtypes

import concourse.bass as bass  # noqa: F401  (bass types used via bacc)
import concourse.mybir as mybir
import concourse.tile as tile
from concourse import bacc
from concourse.bass_utils import run_bass_kernel_spmd

# Problem shapes (hardcoded per spec nn_HGNNP_33629593927812)
N, E, CIN, CH, COUT = 16384, 2048, 1024, 1024, 512
NC = 8                # cores
NL = N // NC          # 2048 local vertices per core
P = 128
KA = CIN // P         # 8 contraction tiles for the W1 matmul
KW = CH // P          # 8 contraction tiles for the W2 matmul
MT = NL // P          # 16 local-vertex tiles
ET = E // P           # 16 edge tiles
CHT = CH // 512       # 2 channel halves of the hidden dim

F8 = mybir.dt.float8e4
F16 = mybir.dt.float16
F32 = mybir.dt.float32
F8NP = ml_dtypes.float8_e4m3   # TRN FP8_EXP4-compatible (max +-240)
RELU = mybir.ActivationFunctionType.Relu
COPY = mybir.ActivationFunctionType.Copy
DR = mybir.MatmulPerfMode.DoubleRow

_CACHE: dict = {}


def _build(with_bias: bool):
    """Build the per-core Bass program (identical on all 8 cores)."""
    nc = bacc.Bacc(None, target_bir_lowering=False, num_devices=NC)

    # Per-core inputs (host-prepared layouts; see kernel() below)
    xt = nc.dram_tensor("xt", [MT, P, KA * P], F8, kind="ExternalInput")
    w1 = nc.dram_tensor("w1", [KA * P, CH], F8, kind="ExternalInput")
    w2 = nc.dram_tensor("w2", [CH, COUT], F16, kind="ExternalInput")
    a_t = nc.dram_tensor("a_t", [ET, P, NL], F8, kind="ExternalInput")
    b_t = nc.dram_tensor("b_t", [E, NL], F8, kind="ExternalInput")
    invde = nc.dram_tensor("invde", [P, ET], F32, kind="ExternalInput")
    invde16 = nc.dram_tensor("invde16", [P, ET], F32, kind="ExternalInput")
    invdv = nc.dram_tensor("invdv", [P, MT], F32, kind="ExternalInput")
    invdv16 = nc.dram_tensor("invdv16", [P, MT], F32, kind="ExternalInput")
    dvf = nc.dram_tensor("dvf", [P, MT], F32, kind="ExternalInput")
    dv_b = nc.dram_tensor("dv_b", [P, NL], F16, kind="ExternalInput")
    mu1d8 = nc.dram_tensor("mu1d8", [P, CH], F16, kind="ExternalInput")
    mu1pp = nc.dram_tensor("mu1pp", [P, KW], F32, kind="ExternalInput")
    mu2b = nc.dram_tensor("mu2b", [P, COUT], F32, kind="ExternalInput")
    if with_bias:
        ones_r = nc.dram_tensor("ones_r", [1, NL], F16, kind="ExternalInput")
        dv_r = nc.dram_tensor("dv_r", [1, NL], F16, kind="ExternalInput")
        b1s_r = nc.dram_tensor("b1s_r", [1, CH], F16, kind="ExternalInput")
        b2_r = nc.dram_tensor("b2_r", [1, COUT], F16, kind="ExternalInput")
    out = nc.dram_tensor("out", [NL, COUT], F32, kind="ExternalOutput")

    RG = [list(range(NC))]
    L2C = [(0, 8), (8, 8)]  # layer-2 AllReduce chunks over edge tiles

    with tile.TileContext(nc) as tc:
        with (
            tc.tile_pool(name="persist", bufs=1) as persist,
            tc.tile_pool(name="stream", bufs=4) as stream,
            tc.tile_pool(name="stage", bufs=6) as stage,
            tc.tile_pool(name="psum", bufs=8, space="PSUM") as psum_pool,
            tc.tile_pool(name="dram", bufs=1, space="DRAM") as dram,
        ):
            # ---- resident weights & scales ----
            # sync ring: w1 then the xt stream (startup critical). Small
            # scale/mean tiles ride the ACT ring ahead of the A/B bulk.
            w1_sb = persist.tile([P, KA, CH], F8, tag="slot_w1")
            w1_v = w1.rearrange("(k pi) c -> pi k c", pi=P)
            nc.sync.dma_start(w1_sb[:, :, 0:512], w1_v[:, :, 0:512])
            nc.sync.dma_start(w1_sb[:, :, 512:1024], w1_v[:, :, 512:1024])
            invde_sb = persist.tile([P, ET], F32)
            invde16_sb = persist.tile([P, ET], F32)
            mu1d8_sb = persist.tile([P, CH], F16)
            nc.scalar.dma_start(invde16_sb[:], invde16[:])
            nc.scalar.dma_start(invde_sb[:], invde[:])
            nc.scalar.dma_start(mu1d8_sb[:], mu1d8[:])
            if with_bias:
                ones_sb = persist.tile([1, NL], F16)
                dvr_sb = persist.tile([1, NL], F16)
                b1s_sb = persist.tile([1, CH], F16)
                b2_sb = persist.tile([1, COUT], F16)
                nc.scalar.dma_start(ones_sb[:], ones_r[:])
                nc.scalar.dma_start(b1s_sb[:], b1s_r[:])
                nc.scalar.dma_start(dvr_sb[:], dv_r[:])
                nc.scalar.dma_start(b2_sb[:], b2_r[:])

            # A (= H_l^T tiles) resident fp8; streamed on the ACT ring during
            # step 1. B (= H^T columns) behind it, needed only at step 3.
            a_sb = persist.tile([P, ET, MT, P], F8, tag="slot_a")  # 4MB
            b_sb = persist.tile([P, ET, NL], F8, tag="slot_b")     # 4MB
            b_v = b_t.rearrange("(po pi) v -> pi po v", pi=P)

            # AllReduce bounce buffers
            e1p_d = [
                dram.tile([E, 512], F8, name=f"e1p_{n}") for n in range(CHT)
            ]
            e1r_d = [
                dram.tile([E, 512], F8, addr_space="Shared", name=f"e1r_{n}")
                for n in range(CHT)
            ]
            e2p_d = [
                dram.tile([nt * P, COUT], F8, name=f"e2p_{h}")
                for h, (_, nt) in enumerate(L2C)
            ]
            e2r_d = [
                dram.tile([nt * P, COUT], F8, addr_space="Shared", name=f"e2r_{h}")
                for h, (_, nt) in enumerate(L2C)
            ]
            e1p_v = [t.rearrange("(po pi) c -> pi po c", pi=P) for t in e1p_d]
            e2p_v = [t.rearrange("(po pi) c -> pi po c", pi=P) for t in e2p_d]

            m1_sb = persist.tile([P, MT, CH], F8, tag="slot_m1")   # 2MB
            e1_sb = persist.tile([P, ET, CH], F8, tag="slot_e1")   # 2MB

            # ---- step 1 (single pass): M1 = relu(X @ W1s) / 32, both
            # channel halves per xt load. A chunks stream behind each xt
            # tile on the ACT ring; the first B chunks ride along (B is
            # needed when step 3 starts, right after AllReduce(c0)).
            for m in range(MT):
                xt_sb = stream.tile([P, KA, P], F8, tag="stream")
                nc.sync.dma_start(xt_sb[:], xt[m])
                nc.scalar.dma_start(a_sb[:, m, :, :], a_t[m])
                if m >= 4:
                    nc.scalar.dma_start(b_sb[:, m - 4, :], b_v[:, m - 4, :])
                for n in range(CHT):
                    cs = slice(n * 512, (n + 1) * 512)
                    ps = psum_pool.tile([P, 512], F32, tag="ps")
                    for k in range(KA // 2):
                        nc.tensor.matmul(
                            ps[:],
                            xt_sb[:, 2 * k:2 * k + 2, :],
                            w1_sb[:, 2 * k:2 * k + 2, cs],
                            start=(k == 0),
                            stop=(k == KA // 2 - 1),
                            perf_mode=DR,
                        )
                    if with_bias:
                        nc.tensor.matmul(
                            ps[:],
                            ones_sb[:, m * P:(m + 1) * P],
                            b1s_sb[:, cs],
                            start=False,
                            stop=True,
                            skip_group_check=True,
                        )
                    nc.scalar.activation(
                        m1_sb[:, m, cs], ps[:], RELU, scale=1.0 / 32.0
                    )

            # ---- step 2, per channel half: E1p = H^T M1 * 16/de - 2*mu1
            for n in range(CHT):
                cs = slice(n * 512, (n + 1) * 512)
                for me in range(ET):
                    if n == 0 and me < 4:
                        nc.scalar.dma_start(
                            b_sb[:, 12 + me, :], b_v[:, 12 + me, :]
                        )
                    ps = psum_pool.tile([P, 512], F32, tag="ps")
                    for k in range(MT // 2):
                        nc.tensor.matmul(
                            ps[:],
                            a_sb[:, me, 2 * k:2 * k + 2, :],
                            m1_sb[:, 2 * k:2 * k + 2, cs],
                            start=(k == 0),
                            stop=(k == MT // 2 - 1),
                            perf_mode=DR,
                        )
                    tm = stage.tile([P, 512], F16, tag="stage")
                    nc.scalar.activation(
                        tm[:], ps[:], COPY, scale=invde16_sb[:, me:me + 1]
                    )
                    st = stage.tile([P, 512], F8, tag="stage8")
                    nc.vector.tensor_sub(st[:], tm[:], mu1d8_sb[:, cs])
                    nc.sync.dma_start(e1p_v[n][:, me, :], st[:])
                nc.gpsimd.collective_compute(
                    "AllReduce",
                    mybir.AluOpType.add,
                    replica_groups=RG,
                    ins=[e1p_d[n].opt()],
                    outs=[e1r_d[n].opt()],
                )

            # E1 residual back to SBUF, directly in fp8. Issued after both
            # collective triggers (a waiting DMA head-of-line-blocks its
            # HWDGE ring). Step-3/4 scale tiles and w2 ride ahead of the
            # collective-gated loads.
            dv_sb = persist.tile([P, NL], F16)
            mu1pp_sb = persist.tile([P, KW], F32)
            invdv_sb = persist.tile([P, MT], F32)
            invdv16_sb = persist.tile([P, MT], F32)
            dvf_sb = persist.tile([P, MT], F32)
            mu2_sb = persist.tile([P, COUT], F32)
            w2_sb = persist.tile([P, KW, COUT], F16, tag="slot_w2")
            nc.sync.dma_start(dv_sb[:], dv_b[:])
            nc.sync.dma_start(mu1pp_sb[:], mu1pp[:])
            nc.sync.dma_start(invdv_sb[:], invdv[:])
            nc.sync.dma_start(invdv16_sb[:], invdv16[:])
            nc.sync.dma_start(dvf_sb[:], dvf[:])
            nc.sync.dma_start(mu2_sb[:], mu2b[:])
            nc.sync.dma_start(w2_sb[:], w2.rearrange("(k pi) c -> pi k c", pi=P))
            for n in range(CHT):
                cs = slice(n * 512, (n + 1) * 512)
                nc.sync.dma_start(
                    e1_sb[:, :, cs],
                    e1r_d[n].rearrange("(po pi) c -> pi po c", pi=P),
                )

            # ---- step 3: V1t = relu(H E1 + dv (x) mu1)^T, [ch, vl] ----
            # output row-block mc only reads E1[:, mc*128:...] => the first
            # half overlaps AllReduce(c1). Rank-1 mean addback on the DVE.
            v1t_sb = persist.tile([P, KW, NL], F16, tag="slot_v1t")  # 4MB
            for mc in range(KW):
                for nv in range(NL // 512):
                    nvs = slice(nv * 512, (nv + 1) * 512)
                    ps = psum_pool.tile([P, 512], F32, tag="ps")
                    for k in range(ET // 2):
                        nc.tensor.matmul(
                            ps[:],
                            e1_sb[:, 2 * k:2 * k + 2, mc * P:(mc + 1) * P],
                            b_sb[:, 2 * k:2 * k + 2, nvs],
                            start=(k == 0),
                            stop=(k == ET // 2 - 1),
                            perf_mode=DR,
                        )
                    r1 = stage.tile([P, 512], F32, tag="stage_r1", bufs=4)
                    nc.vector.tensor_scalar_mul(
                        r1[:], dv_sb[:, nvs], mu1pp_sb[:, mc:mc + 1]
                    )
                    nc.vector.tensor_add(r1[:], ps[:], r1[:])
                    nc.scalar.activation(
                        v1t_sb[:, mc, nvs], r1[:], RELU
                    )

            # ---- step 4: M2r = relu(V1 @ W2) * 1/dv - mu2, fp16 ----
            m2_sb = persist.tile([P, MT, COUT], F8, tag="slot_w1")  # reuse w1
            for m in range(MT):
                ps = psum_pool.tile([P, 512], F32, tag="ps")
                for k in range(KW):
                    nc.tensor.matmul(
                        ps[:],
                        v1t_sb[:, k, m * P:(m + 1) * P],
                        w2_sb[:, k, :],
                        start=(k == 0),
                        stop=(k == KW - 1 and not with_bias),
                    )
                if with_bias:
                    nc.tensor.matmul(
                        ps[:],
                        dvr_sb[:, m * P:(m + 1) * P],
                        b2_sb[:],
                        start=False,
                        stop=True,
                        skip_group_check=True,
                    )
                tm = stage.tile([P, 512], F16, tag="stage_m2", bufs=4)
                nc.scalar.activation(
                    tm[:], ps[:], RELU, scale=invdv_sb[:, m:m + 1]
                )
                nc.vector.tensor_sub(m2_sb[:, m, :], tm[:], mu2_sb[:])

            # ---- step 5: E2p = H^T M2r * 1/de; chunked AllReduce ----
            for h, (t0, nt) in enumerate(L2C):
                for me in range(t0, t0 + nt):
                    ps = psum_pool.tile([P, 512], F32, tag="ps")
                    for k in range(MT // 2):
                        nc.tensor.matmul(
                            ps[:],
                            a_sb[:, me, 2 * k:2 * k + 2, :],
                            m2_sb[:, 2 * k:2 * k + 2, :],
                            start=(k == 0),
                            stop=(k == MT // 2 - 1),
                            perf_mode=DR,
                        )
                    st = stage.tile([P, 512], F8, tag="stage8")
                    nc.scalar.activation(
                        st[:], ps[:], COPY, scale=invde_sb[:, me:me + 1]
                    )
                    nc.sync.dma_start(e2p_v[h][:, me - t0, :], st[:])
                nc.gpsimd.collective_compute(
                    "AllReduce",
                    mybir.AluOpType.add,
                    replica_groups=RG,
                    ins=[e2p_d[h].opt()],
                    outs=[e2r_d[h].opt()],
                )

            e2_sb = persist.tile([P, ET, COUT], F8, tag="slot_w2")  # reuse w2
            for h, (t0, nt) in enumerate(L2C):
                nc.sync.dma_start(
                    e2_sb[:, t0:t0 + nt, :],
                    e2r_d[h].rearrange("(po pi) c -> pi po c", pi=P),
                )

            # ---- step 6: OUT = (H E2 + dv (x) mu2) * 1/dv, [vl, c2] ----
            # Contraction split by AllReduce chunk: chunk-A partials (plus the
            # exact dv*mu2 mean addback) park in SBUF while AllReduce(B) is in
            # flight; a DVE add folds them into the final psum.
            out_v = out.rearrange("(po pi) c -> pi po c", pi=P)
            op_sb = persist.tile([P, MT, COUT], F16, tag="slot_m1")  # reuse m1
            t0a, nta = L2C[0]
            t0b, ntb = L2C[1]
            for m in range(MT):
                ps = psum_pool.tile([P, 512], F32, tag="ps")
                for k in range(t0a // 2, (t0a + nta) // 2):
                    nc.tensor.matmul(
                        ps[:],
                        b_sb[:, 2 * k:2 * k + 2, m * P:(m + 1) * P],
                        e2_sb[:, 2 * k:2 * k + 2, :],
                        start=(k == t0a // 2),
                        stop=(k == (t0a + nta) // 2 - 1),
                        perf_mode=DR,
                    )
                r2 = stage.tile([P, 512], F32, tag="stage_o", bufs=4)
                nc.vector.tensor_scalar_mul(
                    r2[:], mu2_sb[:], dvf_sb[:, m:m + 1]
                )
                nc.vector.tensor_add(op_sb[:, m, :], ps[:], r2[:])
            for m in range(MT):
                ps = psum_pool.tile([P, 512], F32, tag="ps")
                for k in range(t0b // 2, (t0b + ntb) // 2):
                    nc.tensor.matmul(
                        ps[:],
                        b_sb[:, 2 * k:2 * k + 2, m * P:(m + 1) * P],
                        e2_sb[:, 2 * k:2 * k + 2, :],
                        start=(k == t0b // 2),
                        stop=(k == (t0b + ntb) // 2 - 1),
                        perf_mode=DR,
                    )
                st = stage.tile([P, 512], F32, tag="stage_o", bufs=4)
                nc.vector.tensor_add(st[:], ps[:], op_sb[:, m, :])
                so = stage.tile([P, 512], F32, tag="stage_o", bufs=4)
                nc.scalar.activation(
                    so[:], st[:], COPY, scale=invdv16_sb[:, m:m + 1]
                )
                nc.sync.dma_start(out_v[:, m, :], so[:])

    nc.compile()
    return nc


def _q8(x):
    return np.clip(x, -240.0, 240.0).astype(F8NP)


def _prepare_inputs(feature_hyg, pair_v, pair_e, W1, b1, W2, b2, with_bias):
    X = np.ascontiguousarray(np.asarray(feature_hyg, dtype=np.float32))
    pv = np.asarray(pair_v).astype(np.int64)
    pe = np.asarray(pair_e).astype(np.int64)
    W1 = np.asarray(W1, dtype=np.float32)
    b1 = np.asarray(b1, dtype=np.float32)
    W2 = np.asarray(W2, dtype=np.float32)
    b2 = np.asarray(b2, dtype=np.float32)

    ec = np.bincount(pe, minlength=E).astype(np.float32)
    vc = np.bincount(pv, minlength=N).astype(np.float32)
    H = (
        np.bincount(pv * E + pe, minlength=N * E)
        .astype(np.float32)
        .reshape(N, E)
    )
    inv_de = (1.0 / np.maximum(ec, 1.0)).astype(np.float32)
    inv_dv = (1.0 / np.maximum(vc, 1.0)).astype(np.float32)

    # sampled estimate of the M1 column means (the near-rank-one component
    # of E1); mu2 is the induced estimate for M2. Any estimate is *correct*
    # (the decomposition is exact) -- closeness only improves fp8 accuracy.
    rng = np.random.default_rng(12345)
    idx = rng.choice(N, 512, replace=False)
    mu1 = np.maximum(X[idx] @ W1 + b1, 0).mean(axis=0).astype(np.float32)
    mu2 = np.maximum(mu1 @ W2 + b2, 0).astype(np.float32)

    H8 = _q8(H)
    W1q = _q8(W1 * 32.0)
    W2h = W2.astype(np.float16)
    X8 = _q8(X)

    # Residual stages are prescaled x16 to sit mid-range in fp8 (the
    # collectives run in fp8); step-4/6 scales undo it exactly.
    invde_h = np.ascontiguousarray(inv_de.reshape(ET, P).T)
    invde16_h = np.ascontiguousarray(16.0 * invde_h)
    mu1d8_h = np.ascontiguousarray(
        np.broadcast_to((2.0 * mu1).astype(np.float16), (P, CH))
    )
    mu1pp_h = np.ascontiguousarray(16.0 * mu1.reshape(KW, P).T)
    mu2b_h = np.ascontiguousarray(np.broadcast_to(16.0 * mu2, (P, COUT)))

    in_maps = []
    for l in range(NC):
        sl = slice(l * NL, (l + 1) * NL)
        xt_h = np.ascontiguousarray(
            X8[sl].T.reshape(KA, P, MT, P).transpose(2, 1, 0, 3)
        ).reshape(MT, P, KA * P)
        a_h = np.ascontiguousarray(
            H8[sl].reshape(MT, P, ET, P).transpose(2, 1, 0, 3)
        ).reshape(ET, P, MT * P)
        b_h = np.ascontiguousarray(H8[sl].T)
        invdv_h = np.ascontiguousarray(inv_dv[sl].reshape(MT, P).T)
        invdv16_h = np.ascontiguousarray(invdv_h / 16.0)
        dvf_h = np.ascontiguousarray(vc[sl].reshape(MT, P).T)
        dvb_h = np.ascontiguousarray(
            np.broadcast_to(vc[sl].astype(np.float16), (P, NL))
        )
        m = {
            "xt": xt_h, "w1": W1q, "w2": W2h, "a_t": a_h, "b_t": b_h,
            "invde": invde_h, "invde16": invde16_h, "invdv": invdv_h,
            "invdv16": invdv16_h, "dvf": dvf_h,
            "dv_b": dvb_h, "mu1d8": mu1d8_h, "mu1pp": mu1pp_h,
            "mu2b": mu2b_h,
        }
        if with_bias:
            m["ones_r"] = np.ones((1, NL), np.float16)
            m["dv_r"] = vc[sl].astype(np.float16).reshape(1, NL)
            m["b1s_r"] = (32.0 * b1).astype(np.float16).reshape(1, CH)
            m["b2_r"] = (16.0 * b2).astype(np.float16).reshape(1, COUT)
        in_maps.append(m)
    return in_maps


last_result = None  # BassKernelResults of the most recent run (for test harness)


def kernel(feature_hyg, pair_v, pair_e, num_edges, W1, b1, W2, b2):
    global last_result
    assert int(num_edges) == E, f"kernel hardcodes E={E}, got {int(num_edges)}"
    with_bias = bool(np.any(np.asarray(b1)) or np.any(np.asarray(b2)))
    in_maps = _prepare_inputs(
        feature_hyg, pair_v, pair_e, W1, b1, W2, b2, with_bias
    )
    key = ("nc", with_bias)
    if key not in _CACHE:
        _CACHE[key] = _build(with_bias)
    res = run_bass_kernel_spmd(_CACHE[key], in_maps, core_ids=list(range(NC)))
    last_result = res
    out = np.concatenate([res.results[l]["out"] for l in range(NC)], axis=0)
    return np.ascontiguousarray(out.astype(np.float32))


# revision 20
# speedup vs baseline: 1.1138x; 1.1138x over previous
"""HGNN+ (2x HGNNPConv) Trainium2 kernel, 8-core SPMD, fp8 DoubleRow.

Strategy: the hypergraph v2v mean aggregation is a linear operator
    v2v(X) = Dv^-1 H De^-1 H^T X
with H the [N, E] incidence-count matrix. We shard vertices across the
8 cores and run the network as a chain of dense matmuls on the
TensorEngine, with the big contractions in fp8e4 DoubleRow perf mode
(two k-rows per PE cell -> ~2x FLOP rate):

  per core l (NL = N/8 = 2048 local vertices):
    M1  = relu(Xl @ W1) / 32            [NL, CH]   fp8 DR (W1 prescaled x32)
    E1p = H_l^T M1 * 1/de - mu1/8       [E, CH]    fp8 DR + fp32 scales
    E1r = AllReduce(E1p)                [E, CH]    fp8 (x16 residual vs mu1)
    V1t = relu(H E1r + dv (x) mu1)^T    [CH, NL]   fp8 DR + DVE rank-1 addback
    M2r = relu(V1 @ W2)*  1/dv - mu2    [NL, C2]   fp16 matmuls (accuracy)
    E2p = H_l^T M2r * 1/de              [E, C2]    fp8 DR
    E2r = AllReduce(E2p)                [E, C2]    fp8 (x16 residual vs mu2)
    OUT = (H E2r + dv (x) mu2) * 1/dv   [NL, C2]   fp8 DR, fp32 out

Precision design: fp8 RNE quantization of the near-rank-one activations
(E1/M2/E2) produces rounding errors that are correlated across the
segment-mean averaging and would NOT wash out (~3% final error). So H
is kept as raw 0/1 counts (exact in fp8), all 1/de & 1/dv normalizers
are exact fp32 per-partition scales at PSUM->SBUF copies, and E1/M2/E2
are quantized as residuals against a host-sampled mean estimate mu1
(mu2 = relu(mu1@W2)), whose rank-one contribution is carried exactly
and re-added on the (otherwise idle) DVE. W1 is prescaled by 32 so its
entries avoid the fp8 subnormal range. Measured ~2.4e-3 rel err.

Pipelining mirrors the fp16 predecessor: layer 1 runs per 512-wide
channel half with the half-0 AllReduce hidden under half-1 compute and
the half-1 AllReduce hidden under step 3's first output rows; layer 2
chunks its AllReduce over edge-row halves with partials parked in SBUF.
"""

import numpy as np
import ml_dtypes

import concourse.bass as bass  # noqa: F401  (bass types used via bacc)
import concourse.mybir as mybir
import concourse.tile as tile
from concourse import bacc
from concourse.bass_utils import run_bass_kernel_spmd

# Problem shapes (hardcoded per spec nn_HGNNP_33629593927812)
N, E, CIN, CH, COUT = 16384, 2048, 1024, 1024, 512
NC = 8                # cores
NL = N // NC          # 2048 local vertices per core
P = 128
KA = CIN // P         # 8 contraction tiles for the W1 matmul
KW = CH // P          # 8 contraction tiles for the W2 matmul
MT = NL // P          # 16 local-vertex tiles
ET = E // P           # 16 edge tiles
CHT = CH // 512       # 2 channel halves of the hidden dim

F8 = mybir.dt.float8e4
F16 = mybir.dt.float16
F32 = mybir.dt.float32
F8NP = ml_dtypes.float8_e4m3   # TRN FP8_EXP4-compatible (max +-240)
RELU = mybir.ActivationFunctionType.Relu
COPY = mybir.ActivationFunctionType.Copy
DR = mybir.MatmulPerfMode.DoubleRow

_CACHE: dict = {}


def _build(with_bias: bool):
    """Build the per-core Bass program (identical on all 8 cores)."""
    nc = bacc.Bacc(None, target_bir_lowering=False, num_devices=NC)

    # Per-core inputs (host-prepared layouts; see kernel() below)
    xt = nc.dram_tensor("xt", [MT, P, KA * P], F8, kind="ExternalInput")
    w1 = nc.dram_tensor("w1", [KA * P, CH], F8, kind="ExternalInput")
    w2 = nc.dram_tensor("w2", [CH, COUT], F8, kind="ExternalInput")
    a_t = nc.dram_tensor("a_t", [ET, P, NL], F8, kind="ExternalInput")
    b_t = nc.dram_tensor("b_t", [E, NL], F8, kind="ExternalInput")
    invde = nc.dram_tensor("invde", [P, ET], F32, kind="ExternalInput")
    invde16 = nc.dram_tensor("invde16", [P, ET], F32, kind="ExternalInput")
    invdv = nc.dram_tensor("invdv", [P, MT], F32, kind="ExternalInput")
    invdv16 = nc.dram_tensor("invdv16", [P, MT], F32, kind="ExternalInput")
    dvf = nc.dram_tensor("dvf", [P, MT], F32, kind="ExternalInput")
    dv_b = nc.dram_tensor("dv_b", [P, NL], F16, kind="ExternalInput")
    mu1d8 = nc.dram_tensor("mu1d8", [P, CH], F16, kind="ExternalInput")
    mu1pp = nc.dram_tensor("mu1pp", [P, KW], F32, kind="ExternalInput")
    mu2b = nc.dram_tensor("mu2b", [P, COUT], F32, kind="ExternalInput")
    mu1w2b = nc.dram_tensor("mu1w2b", [P, COUT], F32, kind="ExternalInput")
    invdv32 = nc.dram_tensor("invdv32", [P, MT], F32, kind="ExternalInput")
    if with_bias:
        ones_r = nc.dram_tensor("ones_r", [1, NL], F16, kind="ExternalInput")
        b1s_r = nc.dram_tensor("b1s_r", [1, CH], F16, kind="ExternalInput")
    out = nc.dram_tensor("out", [NL, COUT], F32, kind="ExternalOutput")

    RG = [list(range(NC))]
    L2C = [(0, 8), (8, 8)]  # layer-2 AllReduce chunks over edge tiles

    with tile.TileContext(nc) as tc:
        with (
            tc.tile_pool(name="persist", bufs=1) as persist,
            tc.tile_pool(name="stream", bufs=4) as stream,
            tc.tile_pool(name="stage", bufs=6) as stage,
            tc.tile_pool(name="psum", bufs=8, space="PSUM") as psum_pool,
            tc.tile_pool(name="dram", bufs=1, space="DRAM") as dram,
        ):
            # ---- resident weights & scales ----
            # sync ring: w1 then the xt stream (startup critical). Small
            # scale/mean tiles ride the ACT ring ahead of the A/B bulk.
            w1_sb = persist.tile([P, KA, CH], F8, tag="slot_w1")
            w1_v = w1.rearrange("(k pi) c -> pi k c", pi=P)
            nc.sync.dma_start(w1_sb[:, :, 0:512], w1_v[:, :, 0:512])
            nc.sync.dma_start(w1_sb[:, :, 512:1024], w1_v[:, :, 512:1024])
            invde_sb = persist.tile([P, ET], F32)
            invde16_sb = persist.tile([P, ET], F32)
            mu1d8_sb = persist.tile([P, CH], F16)
            nc.scalar.dma_start(invde16_sb[:], invde16[:])
            nc.scalar.dma_start(invde_sb[:], invde[:])
            nc.scalar.dma_start(mu1d8_sb[:], mu1d8[:])
            if with_bias:
                ones_sb = persist.tile([1, NL], F16)
                b1s_sb = persist.tile([1, CH], F16)
                nc.scalar.dma_start(ones_sb[:], ones_r[:])
                nc.scalar.dma_start(b1s_sb[:], b1s_r[:])

            # A (= H_l^T tiles) resident fp8; streamed on the ACT ring during
            # step 1. B (= H^T columns) behind it, needed only at step 3.
            a_sb = persist.tile([P, ET, MT, P], F8, tag="slot_a")  # 4MB
            b_sb = persist.tile([P, ET, NL], F8, tag="slot_b")     # 4MB
            b_v = b_t.rearrange("(po pi) v -> pi po v", pi=P)

            # AllReduce bounce buffers
            e1p_d = [
                dram.tile([E, 512], F8, name=f"e1p_{n}") for n in range(CHT)
            ]
            e1r_d = [
                dram.tile([E, 512], F8, addr_space="Shared", name=f"e1r_{n}")
                for n in range(CHT)
            ]
            e2p_d = [
                dram.tile([nt * P, COUT], F8, name=f"e2p_{h}")
                for h, (_, nt) in enumerate(L2C)
            ]
            e2r_d = [
                dram.tile([nt * P, COUT], F8, addr_space="Shared", name=f"e2r_{h}")
                for h, (_, nt) in enumerate(L2C)
            ]
            e1p_v = [t.rearrange("(po pi) c -> pi po c", pi=P) for t in e1p_d]
            e2p_v = [t.rearrange("(po pi) c -> pi po c", pi=P) for t in e2p_d]

            # Warm-up collective: the first collective on a NEFF pays ~10us
            # of one-time setup and absorbs any inter-core start skew. Fire
            # a tiny throwaway AllReduce immediately so AllReduce(c0) later
            # starts within ~2us of its trigger. Result is never read.
            warm_d = dram.tile([P, ET], F32, name="warm_d")
            warm_r = dram.tile(
                [P, ET], F32, addr_space="Shared", name="warm_r"
            )
            nc.scalar.dma_start(warm_d[:], invde[:])
            nc.gpsimd.collective_compute(
                "AllReduce",
                mybir.AluOpType.add,
                replica_groups=RG,
                ins=[warm_d.opt()],
                outs=[warm_r.opt()],
            )

            m1_sb = persist.tile([P, MT, CH], F8, tag="slot_m1")   # 2MB
            e1_sb = persist.tile([P, ET, CH], F8, tag="slot_e1")   # 2MB

            # ---- step 1 (single pass): M1 = relu(X @ W1s) / 32, both
            # channel halves per xt load. A chunks stream behind each xt
            # tile on the ACT ring; the first B chunks ride along (B is
            # needed when step 3 starts, right after AllReduce(c0)).
            for m in range(MT):
                xt_sb = stream.tile([P, KA, P], F8, tag="stream")
                nc.sync.dma_start(xt_sb[:], xt[m])
                nc.scalar.dma_start(a_sb[:, m, :, :], a_t[m])
                if m >= 4:
                    nc.scalar.dma_start(b_sb[:, m - 4, :], b_v[:, m - 4, :])
                for n in range(CHT):
                    cs = slice(n * 512, (n + 1) * 512)
                    ps = psum_pool.tile([P, 512], F32, tag="ps")
                    for k in range(KA // 2):
                        nc.tensor.matmul(
                            ps[:],
                            xt_sb[:, 2 * k:2 * k + 2, :],
                            w1_sb[:, 2 * k:2 * k + 2, cs],
                            start=(k == 0),
                            stop=(k == KA // 2 - 1),
                            perf_mode=DR,
                        )
                    if with_bias:
                        nc.tensor.matmul(
                            ps[:],
                            ones_sb[:, m * P:(m + 1) * P],
                            b1s_sb[:, cs],
                            start=False,
                            stop=True,
                            skip_group_check=True,
                        )
                    nc.scalar.activation(
                        m1_sb[:, m, cs], ps[:], RELU, scale=1.0 / 32.0
                    )

            # ---- step 2, per channel half: E1p = H^T M1 * 16/de - 2*mu1
            for n in range(CHT):
                cs = slice(n * 512, (n + 1) * 512)
                for me in range(ET):
                    if n == 0 and me < 4:
                        nc.scalar.dma_start(
                            b_sb[:, 12 + me, :], b_v[:, 12 + me, :]
                        )
                    ps = psum_pool.tile([P, 512], F32, tag="ps")
                    for k in range(MT // 2):
                        nc.tensor.matmul(
                            ps[:],
                            a_sb[:, me, 2 * k:2 * k + 2, :],
                            m1_sb[:, 2 * k:2 * k + 2, cs],
                            start=(k == 0),
                            stop=(k == MT // 2 - 1),
                            perf_mode=DR,
                        )
                    tm = stage.tile([P, 512], F16, tag="stage")
                    nc.scalar.activation(
                        tm[:], ps[:], COPY, scale=invde16_sb[:, me:me + 1]
                    )
                    st = stage.tile([P, 512], F8, tag="stage8")
                    nc.vector.tensor_sub(st[:], tm[:], mu1d8_sb[:, cs])
                    nc.sync.dma_start(e1p_v[n][:, me, :], st[:])
                nc.gpsimd.collective_compute(
                    "AllReduce",
                    mybir.AluOpType.add,
                    replica_groups=RG,
                    ins=[e1p_d[n].opt()],
                    outs=[e1r_d[n].opt()],
                )

            # E1 residual back to SBUF, directly in fp8. Issued after both
            # collective triggers (a waiting DMA head-of-line-blocks its
            # HWDGE ring). Step-3/4 scale tiles and w2 ride ahead of the
            # collective-gated loads.
            dv_sb = persist.tile([P, NL], F16)
            mu1pp_sb = persist.tile([P, KW], F32)
            invdv_sb = persist.tile([P, MT], F32)
            invdv16_sb = persist.tile([P, MT], F32)
            dvf_sb = persist.tile([P, MT], F32)
            mu2_sb = persist.tile([P, COUT], F32)
            w2_sb = persist.tile([P, KW, COUT], F8, tag="slot_w2")
            nc.sync.dma_start(dv_sb[:], dv_b[:])
            nc.sync.dma_start(mu1pp_sb[:], mu1pp[:])
            nc.sync.dma_start(invdv_sb[:], invdv[:])
            nc.sync.dma_start(invdv16_sb[:], invdv16[:])
            invdv32_sb = persist.tile([P, MT], F32)
            mu1w2_sb = persist.tile([P, COUT], F32)
            nc.sync.dma_start(invdv32_sb[:], invdv32[:])
            nc.sync.dma_start(mu1w2_sb[:], mu1w2b[:])
            nc.sync.dma_start(dvf_sb[:], dvf[:])
            nc.sync.dma_start(mu2_sb[:], mu2b[:])
            nc.sync.dma_start(w2_sb[:], w2.rearrange("(k pi) c -> pi k c", pi=P))
            for n in range(CHT):
                cs = slice(n * 512, (n + 1) * 512)
                nc.sync.dma_start(
                    e1_sb[:, :, cs],
                    e1r_d[n].rearrange("(po pi) c -> pi po c", pi=P),
                )

            # ---- step 3: V1res = relu(H E1 + dv (x) mu1) - dv (x) mu1 ----
            # [ch, vl] layout; output row-block mc only reads E1[:, mc*128:]
            # => the first half overlaps AllReduce(c1). Rank-1 mean addback
            # (and re-subtract, so step 4 runs fp8 on the residual) on DVE.
            v1t_sb = persist.tile([P, KW, NL], F8, tag="slot_v1t")  # 2MB
            for mc in range(KW):
                for nv in range(NL // 512):
                    nvs = slice(nv * 512, (nv + 1) * 512)
                    ps = psum_pool.tile([P, 512], F32, tag="ps")
                    for k in range(ET // 2):
                        nc.tensor.matmul(
                            ps[:],
                            e1_sb[:, 2 * k:2 * k + 2, mc * P:(mc + 1) * P],
                            b_sb[:, 2 * k:2 * k + 2, nvs],
                            start=(k == 0),
                            stop=(k == ET // 2 - 1),
                            perf_mode=DR,
                        )
                    rk = stage.tile([P, 512], F32, tag="stage_r1", bufs=4)
                    nc.vector.tensor_scalar_mul(
                        rk[:], dv_sb[:, nvs], mu1pp_sb[:, mc:mc + 1]
                    )
                    t3 = stage.tile([P, 512], F32, tag="stage_t", bufs=4)
                    nc.vector.tensor_add(t3[:], ps[:], rk[:])
                    u3 = stage.tile([P, 512], F16, tag="stage_u", bufs=4)
                    nc.scalar.activation(u3[:], t3[:], RELU)
                    nc.vector.tensor_sub(v1t_sb[:, mc, nvs], u3[:], rk[:])

            # ---- step 4: M2r = relu((V1res + dv (x) mu1) @ W2) / dv - mu2,
            # fp8 DoubleRow on the residual; the exact rank-1 term
            # dv (x) (mu1 @ W2 + b2) is re-added on the DVE pre-relu.
            m2_sb = persist.tile([P, MT, COUT], F8, tag="slot_w1")  # reuse w1
            for m in range(MT):
                ps = psum_pool.tile([P, 512], F32, tag="ps")
                for k in range(KW // 2):
                    nc.tensor.matmul(
                        ps[:],
                        v1t_sb[:, 2 * k:2 * k + 2, m * P:(m + 1) * P],
                        w2_sb[:, 2 * k:2 * k + 2, :],
                        start=(k == 0),
                        stop=(k == KW // 2 - 1),
                        perf_mode=DR,
                    )
                rk4 = stage.tile([P, 512], F32, tag="stage_m2f", bufs=4)
                nc.vector.tensor_scalar_mul(
                    rk4[:], mu1w2_sb[:], dvf_sb[:, m:m + 1]
                )
                nc.vector.tensor_add(rk4[:], ps[:], rk4[:])
                tm = stage.tile([P, 512], F16, tag="stage_m2", bufs=4)
                nc.scalar.activation(
                    tm[:], rk4[:], RELU, scale=invdv32_sb[:, m:m + 1]
                )
                nc.vector.tensor_sub(m2_sb[:, m, :], tm[:], mu2_sb[:])

            # ---- step 5: E2p = H^T M2r * 1/de; chunked AllReduce ----
            for h, (t0, nt) in enumerate(L2C):
                for me in range(t0, t0 + nt):
                    ps = psum_pool.tile([P, 512], F32, tag="ps")
                    for k in range(MT // 2):
                        nc.tensor.matmul(
                            ps[:],
                            a_sb[:, me, 2 * k:2 * k + 2, :],
                            m2_sb[:, 2 * k:2 * k + 2, :],
                            start=(k == 0),
                            stop=(k == MT // 2 - 1),
                            perf_mode=DR,
                        )
                    st = stage.tile([P, 512], F8, tag="stage8")
                    nc.scalar.activation(
                        st[:], ps[:], COPY, scale=invde_sb[:, me:me + 1]
                    )
                    nc.sync.dma_start(e2p_v[h][:, me - t0, :], st[:])
                nc.gpsimd.collective_compute(
                    "AllReduce",
                    mybir.AluOpType.add,
                    replica_groups=RG,
                    ins=[e2p_d[h].opt()],
                    outs=[e2r_d[h].opt()],
                )

            e2_sb = persist.tile([P, ET, COUT], F8, tag="slot_w2")  # reuse w2
            for h, (t0, nt) in enumerate(L2C):
                nc.sync.dma_start(
                    e2_sb[:, t0:t0 + nt, :],
                    e2r_d[h].rearrange("(po pi) c -> pi po c", pi=P),
                )

            # ---- step 6: OUT = (H E2 + dv (x) mu2) * 1/dv, [vl, c2] ----
            # Contraction split by AllReduce chunk: chunk-A partials (plus the
            # exact dv*mu2 mean addback) park in SBUF while AllReduce(B) is in
            # flight; a DVE add folds them into the final psum.
            out_v = out.rearrange("(po pi) c -> pi po c", pi=P)
            op_sb = persist.tile([P, MT, COUT], F16, tag="slot_m1")  # reuse m1
            t0a, nta = L2C[0]
            t0b, ntb = L2C[1]
            for m in range(MT):
                ps = psum_pool.tile([P, 512], F32, tag="ps")
                for k in range(t0a // 2, (t0a + nta) // 2):
                    nc.tensor.matmul(
                        ps[:],
                        b_sb[:, 2 * k:2 * k + 2, m * P:(m + 1) * P],
                        e2_sb[:, 2 * k:2 * k + 2, :],
                        start=(k == t0a // 2),
                        stop=(k == (t0a + nta) // 2 - 1),
                        perf_mode=DR,
                    )
                r2 = stage.tile([P, 512], F32, tag="stage_o", bufs=4)
                nc.vector.tensor_scalar_mul(
                    r2[:], mu2_sb[:], dvf_sb[:, m:m + 1]
                )
                nc.vector.tensor_add(op_sb[:, m, :], ps[:], r2[:])
            for m in range(MT):
                ps = psum_pool.tile([P, 512], F32, tag="ps")
                for k in range(t0b // 2, (t0b + ntb) // 2):
                    nc.tensor.matmul(
                        ps[:],
                        b_sb[:, 2 * k:2 * k + 2, m * P:(m + 1) * P],
                        e2_sb[:, 2 * k:2 * k + 2, :],
                        start=(k == t0b // 2),
                        stop=(k == (t0b + ntb) // 2 - 1),
                        perf_mode=DR,
                    )
                st = stage.tile([P, 512], F32, tag="stage_o", bufs=4)
                nc.vector.tensor_add(st[:], ps[:], op_sb[:, m, :])
                so = stage.tile([P, 512], F32, tag="stage_o", bufs=4)
                nc.scalar.activation(
                    so[:], st[:], COPY, scale=invdv16_sb[:, m:m + 1]
                )
                eng = nc.sync if m % 2 == 0 else nc.scalar
                eng.dma_start(out_v[:, m, :], so[:])

    nc.compile()
    return nc


def _q8(x):
    return np.clip(x, -240.0, 240.0).astype(F8NP)


def _prepare_inputs(feature_hyg, pair_v, pair_e, W1, b1, W2, b2, with_bias):
    X = np.ascontiguousarray(np.asarray(feature_hyg, dtype=np.float32))
    pv = np.asarray(pair_v).astype(np.int64)
    pe = np.asarray(pair_e).astype(np.int64)
    W1 = np.asarray(W1, dtype=np.float32)
    b1 = np.asarray(b1, dtype=np.float32)
    W2 = np.asarray(W2, dtype=np.float32)
    b2 = np.asarray(b2, dtype=np.float32)

    ec = np.bincount(pe, minlength=E).astype(np.float32)
    vc = np.bincount(pv, minlength=N).astype(np.float32)
    H = (
        np.bincount(pv * E + pe, minlength=N * E)
        .astype(np.float32)
        .reshape(N, E)
    )
    inv_de = (1.0 / np.maximum(ec, 1.0)).astype(np.float32)
    inv_dv = (1.0 / np.maximum(vc, 1.0)).astype(np.float32)

    # sampled estimate of the M1 column means (the near-rank-one component
    # of E1); mu2 is the induced estimate for M2. Any estimate is *correct*
    # (the decomposition is exact) -- closeness only improves fp8 accuracy.
    rng = np.random.default_rng(12345)
    idx = rng.choice(N, 512, replace=False)
    mu1 = np.maximum(X[idx] @ W1 + b1, 0).mean(axis=0).astype(np.float32)
    mu2 = np.maximum(mu1 @ W2 + b2, 0).astype(np.float32)

    H8 = _q8(H)
    W1q = _q8(W1 * 32.0)
    W2q = _q8(W2 * 32.0)
    X8 = _q8(X)

    # Residual stages are prescaled x16 to sit mid-range in fp8 (the
    # collectives run in fp8); step-4/6 scales undo it exactly.
    invde_h = np.ascontiguousarray(inv_de.reshape(ET, P).T)
    invde16_h = np.ascontiguousarray(16.0 * invde_h)
    mu1d8_h = np.ascontiguousarray(
        np.broadcast_to((2.0 * mu1).astype(np.float16), (P, CH))
    )
    mu1pp_h = np.ascontiguousarray(16.0 * mu1.reshape(KW, P).T)
    mu2b_h = np.ascontiguousarray(np.broadcast_to(16.0 * mu2, (P, COUT)))
    mu1w2_h = np.ascontiguousarray(
        np.broadcast_to(512.0 * (mu1 @ W2 + b2).astype(np.float32), (P, COUT))
    )

    in_maps = []
    for l in range(NC):
        sl = slice(l * NL, (l + 1) * NL)
        xt_h = np.ascontiguousarray(
            X8[sl].T.reshape(KA, P, MT, P).transpose(2, 1, 0, 3)
        ).reshape(MT, P, KA * P)
        a_h = np.ascontiguousarray(
            H8[sl].reshape(MT, P, ET, P).transpose(2, 1, 0, 3)
        ).reshape(ET, P, MT * P)
        b_h = np.ascontiguousarray(H8[sl].T)
        invdv_h = np.ascontiguousarray(inv_dv[sl].reshape(MT, P).T)
        invdv16_h = np.ascontiguousarray(invdv_h / 16.0)
        invdv32_h = np.ascontiguousarray(invdv_h / 32.0)
        dvf_h = np.ascontiguousarray(vc[sl].reshape(MT, P).T)
        dvb_h = np.ascontiguousarray(
            np.broadcast_to(vc[sl].astype(np.float16), (P, NL))
        )
        m = {
            "xt": xt_h, "w1": W1q, "w2": W2q, "a_t": a_h, "b_t": b_h,
            "invde": invde_h, "invde16": invde16_h, "invdv": invdv_h,
            "invdv16": invdv16_h, "invdv32": invdv32_h, "dvf": dvf_h,
            "dv_b": dvb_h, "mu1d8": mu1d8_h, "mu1pp": mu1pp_h,
            "mu2b": mu2b_h, "mu1w2b": mu1w2_h,
        }
        if with_bias:
            m["ones_r"] = np.ones((1, NL), np.float16)
            m["b1s_r"] = (32.0 * b1).astype(np.float16).reshape(1, CH)
        in_maps.append(m)
    return in_maps


last_result = None  # BassKernelResults of the most recent run (for test harness)


def kernel(feature_hyg, pair_v, pair_e, num_edges, W1, b1, W2, b2):
    global last_result
    assert int(num_edges) == E, f"kernel hardcodes E={E}, got {int(num_edges)}"
    with_bias = bool(np.any(np.asarray(b1)) or np.any(np.asarray(b2)))
    in_maps = _prepare_inputs(
        feature_hyg, pair_v, pair_e, W1, b1, W2, b2, with_bias
    )
    key = ("nc", with_bias)
    if key not in _CACHE:
        _CACHE[key] = _build(with_bias)
    res = run_bass_kernel_spmd(_CACHE[key], in_maps, core_ids=list(range(NC)))
    last_result = res
    out = np.concatenate([res.results[l]["out"] for l in range(NC)], axis=0)
    return np.ascontiguousarray(out.astype(np.float32))
